# revision 8
# baseline (speedup 1.0000x reference)
"""Causal multi-head attention on 8 Trainium2 NeuronCores.

Problem: B=2, S=2048, E=1024, H=16 heads, D=64.
Sharding: core c handles batch b = c // 4 and heads [4*(c%4) .. 4*(c%4)+3]
(data parallel on B x tensor parallel on heads). Each core computes its
partial output projection in bf16; the host sums the 4 partials per batch
and adds b_proj (standard row-parallel TP reduction, done on host in f64).

Per-core kernel (all matmuls bf16 inputs, fp32 PSUM accumulation), computed
in "transposed" space to avoid transposing softmax probabilities:
  xT = X[b]^T in SBUF.
  Q^T/K^T [D, S] per head via col-packed matmuls (Q in psum rows 0:64, K in
  rows 64:128); all 8 Q k-tiles issued before the 8 K k-tiles so the PE
  stays in one array configuration per group.
  V [S, D] per head with a ones column appended -> AV matmul also
  accumulates softmax denominators in psum row 64.
  Scores S^T [k, q] = K^T.T @ Q^T, row-packed across head pairs; emitted in
  two-block groups so consecutive score pairs share the K=64 PE config.
  P^T = exp(S^T) on ACT; diagonal blocks masked with a precomputed causal
  strip mask on gpsimd; strictly-upper blocks skipped.
  A^T [65, q] = V_ext.T @ P^T accumulated over k tiles (row 64 = denom).
  Per-pair finalize all on DVE: evict raw A^T+denoms (fp32), reciprocal to
  bf16, one K=2 matmul broadcasts both heads' 1/denom across partitions,
  tensor_mul divides into pair-stacked A_scaled^T.
  partial[q, e] = A_scaled^T.T @ W_proj_rows accumulated over both pairs,
  with the stationary operand reused across both 512-wide e halves;
  evicted bf16 and DMA'd out.
Filler (next macro's QKV units, previous macro's projection) is emitted
paced into the j-loops so the PE never drains at pair/macro seams.
"""

import os
import sys
from collections import deque
from contextlib import ExitStack

for _p in ("/opt/trn_rl_repo", "/root/.axon_site/_ro/trn_rl_repo"):
    if os.path.isdir(_p) and _p not in sys.path:
        sys.path.append(_p)

import numpy as np
import ml_dtypes

import concourse.bass as bass
import concourse.tile as tile
from concourse import bacc
from concourse import mybir
from concourse.masks import make_identity  # noqa: F401  (import check)

FP32 = mybir.dt.float32
BF16 = mybir.dt.bfloat16
AF = mybir.ActivationFunctionType

B, S, E, H = 2, 2048, 1024, 16
D = E // H          # 64
NCORES = 8
HPC = 4             # heads per core
NPAIR = 2           # head pairs per core
KT = E // 128       # 8 contraction tiles over E
ST = S // 128       # 16 tiles over S (k dimension)
QM = S // 512       # 4 q-macro tiles of 512
NQ = 512


def build_graph():
    nc = bacc.Bacc()

    xT = nc.declare_dram_parameter("xT", [E, S], BF16, isOutput=False)
    wq = nc.declare_dram_parameter("wq", [E, HPC * D], BF16, isOutput=False)
    wk = nc.declare_dram_parameter("wk", [E, HPC * D], BF16, isOutput=False)
    wv = nc.declare_dram_parameter("wv", [E, HPC * D], BF16, isOutput=False)
    qkbias = nc.declare_dram_parameter("qkbias", [128, HPC], FP32, isOutput=False)
    wp = nc.declare_dram_parameter("wp", [HPC * D, E], BF16, isOutput=False)
    out = nc.declare_dram_parameter("out", [S, E], BF16, isOutput=True)

    with tile.TileContext(nc) as tc, ExitStack() as ctx:
        const = ctx.enter_context(tc.tile_pool(name="const", bufs=1))
        sb = ctx.enter_context(tc.tile_pool(name="sb", bufs=1))
        pexp_pool = ctx.enter_context(tc.tile_pool(name="pexp", bufs=10))
        stage = ctx.enter_context(tc.tile_pool(name="stage", bufs=4))
        rec_pool = ctx.enter_context(tc.tile_pool(name="rec", bufs=2))
        araw_pool = ctx.enter_context(tc.tile_pool(name="araw", bufs=2))

        # PSUM budget is 8 banks: scores 2x2-bank + qkv/proj/bcast 2 + psa 2
        ps_s = ctx.enter_context(tc.tile_pool(name="ps_s", bufs=2, space="PSUM"))
        ps_qkv = ctx.enter_context(tc.tile_pool(name="ps_qkv", bufs=2, space="PSUM"))
        ps_a = ctx.enter_context(tc.tile_pool(name="ps_a", bufs=1, space="PSUM"))

        # ---- persistent SBUF tensors ----
        xt_sb = sb.tile([128, KT, S], BF16)          # X^T tiles, kt-major
        qt_sb = sb.tile([128, NPAIR, S], BF16)       # Q^T, pair-stacked
        kt_sb = sb.tile([128, NPAIR, S], BF16)       # K^T, pair-stacked
        v_sb = sb.tile([128, ST, HPC, D + 1], BF16)  # [V | ones] per ktile/head
        as_sb = sb.tile([128, NPAIR, S], BF16)       # A_scaled^T, pair-stacked
        wq_sb = sb.tile([128, KT, HPC * D], BF16)
        wk_sb = sb.tile([128, KT, HPC * D], BF16)
        wv_sb = sb.tile([128, KT, HPC * D], BF16)
        wp_sb = sb.tile([128, NPAIR, E], BF16)
        qkb_sb = const.tile([128, HPC], FP32)
        ones33 = const.tile([33, 128], BF16)         # K=33 lhsT: bcast both heads
        tri = const.tile([128, 128], BF16)           # upper-tri (incl diag) strip mask

        # ---- constants ----
        # ones33: row 0 -> head 0's output partitions 0:64, row 32 -> head
        # 1's partitions 64:128 (32-aligned partition bases only).
        nc.any.memset(ones33[:], 0.0)
        nc.any.memset(ones33[0:1, 0:64], 1.0)
        nc.any.memset(ones33[32:33, 64:128], 1.0)
        nc.any.memset(v_sb[:, :, :, D:D + 1], 1.0)
        # PE warm-up: dummy matmuls on a zero tile while input DMAs land.
        warm = const.tile([128, NQ], BF16)
        nc.vector.memset(warm[:], 0.0)
        psw = ps_s.tile([128, 2 * NQ], FP32, name="psw", tag="ss")
        for _w in range(12):
            nc.tensor.matmul(psw[:, 0:NQ], lhsT=warm[:, 0:128], rhs=warm[:],
                             start=(_w == 0), stop=(_w == 11))
        # tri[kk, qq] = 1 where kk <= qq else 0
        nc.any.memset(tri[:], 1.0)
        nc.gpsimd.affine_select(
            out=tri[:], in_=tri[:],
            compare_op=mybir.AluOpType.is_ge, fill=0.0,
            base=0, pattern=[[1, 128]], channel_multiplier=-1)

        # ---- input DMAs: critical-first, alternating the two HWDGE rings ----
        _dq = [nc.sync, nc.scalar]
        _di = [0]

        def dma_in(dst, src):
            _dq[_di[0] % 2].dma_start(dst, src)
            _di[0] += 1

        for kt in range(KT):
            dma_in(wq_sb[:, kt, :], wq[kt * 128:(kt + 1) * 128, :])
            dma_in(xt_sb[:, kt, 0:NQ], xT[kt * 128:(kt + 1) * 128, 0:NQ])
        dma_in(qkb_sb[:], qkbias[:])
        for kt in range(KT):
            dma_in(wk_sb[:, kt, :], wk[kt * 128:(kt + 1) * 128, :])
        for kt in range(KT):
            dma_in(wv_sb[:, kt, :], wv[kt * 128:(kt + 1) * 128, :])
        for kt in range(KT):
            dma_in(xt_sb[:, kt, NQ:2 * NQ], xT[kt * 128:(kt + 1) * 128, NQ:2 * NQ])
        dma_in(wp_sb[:, 0, :], wp[0:128, :])
        dma_in(wp_sb[:, 1, :], wp[128:256, :])
        for _c in range(2, QM):
            for kt in range(KT):
                dma_in(xt_sb[:, kt, _c * NQ:(_c + 1) * NQ],
                       xT[kt * 128:(kt + 1) * 128, _c * NQ:(_c + 1) * NQ])

        # ---- filler unit builders ----
        def v_unit(st):
            def emit():
                psv = ps_qkv.tile([128, NQ], FP32, name="psv", tag="qkv")
                ssl = slice(st * 128, (st + 1) * 128)
                for kt in range(KT):
                    nc.tensor.matmul(
                        psv[:, 0:HPC * D], lhsT=xt_sb[:, kt, ssl],
                        rhs=wv_sb[:, kt, :], start=(kt == 0),
                        stop=(kt == KT - 1))
                nc.vector.tensor_copy(
                    v_sb[:, st, :, 0:D],
                    psv[:, 0:HPC * D].rearrange("p (h d) -> p h d", h=HPC))
            return emit

        def qk_unit(mm, p, hh):
            # all 8 Q k-tiles, then all 8 K k-tiles: one PE config switch
            def emit():
                msl = slice(mm * NQ, (mm + 1) * NQ)
                h = 2 * p + hh
                lo, hi = hh * 64, hh * 64 + 64
                psqk = ps_qkv.tile([128, NQ], FP32, name="psqk", tag="qkv")
                for kt in range(KT):
                    nc.tensor.matmul(
                        psqk[0:64, :],
                        lhsT=wq_sb[:, kt, h * D:(h + 1) * D],
                        rhs=xt_sb[:, kt, msl],
                        start=(kt == 0), stop=(kt == KT - 1),
                        tile_position=(0, 0), skip_group_check=True)
                for kt in range(KT):
                    nc.tensor.matmul(
                        psqk[64:128, :],
                        lhsT=wk_sb[:, kt, h * D:(h + 1) * D],
                        rhs=xt_sb[:, kt, msl],
                        start=(kt == 0), stop=(kt == KT - 1),
                        tile_position=(0, 64), skip_group_check=True)
                nc.vector.tensor_scalar(
                    qt_sb[lo:hi, p, msl], psqk[0:64, :],
                    0.125, qkb_sb[0:64, h:h + 1],
                    op0=mybir.AluOpType.mult, op1=mybir.AluOpType.add)
                nc.vector.tensor_scalar_add(
                    kt_sb[lo:hi, p, msl], psqk[64:128, :],
                    qkb_sb[64:128, h:h + 1])
            return emit

        def proj_unit(mm, t):
            # stationary operand (as_sb pair block) reused for both e halves
            def emit():
                tsl = slice(mm * NQ + t * 128, mm * NQ + (t + 1) * 128)
                pso = [ps_qkv.tile([128, 512], FP32, name=f"pso{e}", tag="qkv")
                       for e in range(2)]
                for pp in range(NPAIR):
                    for e in range(2):
                        nc.tensor.matmul(
                            pso[e][:], lhsT=as_sb[:, pp, tsl],
                            rhs=wp_sb[:, pp, e * 512:(e + 1) * 512],
                            start=(pp == 0), stop=(pp == NPAIR - 1))
                for e in range(2):
                    osb = stage.tile([128, 512], BF16)
                    nc.vector.tensor_copy(osb[:], pso[e][:])
                    nc.sync.dma_start(out[tsl, e * 512:(e + 1) * 512], osb[:])
            return emit

        def make_fb(m, p, araw, recb, psb_tag):
            # broadcast 1/denom for both heads with one K=2 matmul, divide
            def emit():
                msl = slice(m * NQ, (m + 1) * NQ)
                psb = ps_qkv.tile([128, NQ], FP32, name="psb", tag=psb_tag) \
                    if psb_tag == "qkv" else \
                    ps_s.tile([128, NQ], FP32, name="psbs", tag="ss")
                nc.tensor.matmul(psb[:, :], lhsT=ones33[:], rhs=recb[0:33, :],
                                 start=True, stop=True)
                for hh in range(2):
                    lo, hi = hh * 64, hh * 64 + 64
                    nc.vector.tensor_mul(
                        as_sb[lo:hi, p, msl], psb[lo:hi, :], araw[hh][0:64, :])
            return emit

        fill_q = deque()

        def pop_fill(k):
            while k > 0 and fill_q:
                fill_q.popleft()()
                k -= 1

        # ---- macro 0 prologue: pair-0 QK while DMAs land, then V 0..3 ----
        qk_unit(0, 0, 0)()
        qk_unit(0, 0, 1)()
        for st in range(4):
            v_unit(st)()
        fill_q.append(qk_unit(0, 1, 0))
        fill_q.append(qk_unit(0, 1, 1))

        for m in range(QM):
            nblk = 4 * m + 4
            msl_q = slice(m * NQ, (m + 1) * NQ)
            gidx = 0
            for p in range(NPAIR):
                psa = [ps_a.tile([65, NQ], FP32, name=f"psa{_hh}",
                                 tag=f"psa{_hh}") for _hh in range(2)]
                for jj in range(0, nblk, 2):
                    infos = []
                    for j in (jj, jj + 1):
                        jsl = slice(j * 128, (j + 1) * 128)
                        r = j - 4 * m
                        c0 = 128 * r if r > 0 else 0
                        pss = ps_s.tile([128, 2 * NQ], FP32, name="pss",
                                        tag="ss")
                        for hh in range(2):
                            lo, hi = hh * 64, hh * 64 + 64
                            nc.tensor.matmul(
                                pss[:, hh * NQ + c0:hh * NQ + NQ],
                                lhsT=kt_sb[lo:hi, p, jsl],
                                rhs=qt_sb[lo:hi, p,
                                          m * NQ + c0:m * NQ + NQ],
                                start=True, stop=True,
                                tile_position=(hh * 64, 0))
                        infos.append((j, pss, c0, r))
                    pes = []
                    for j, pss, c0, r in infos:
                        pe = pexp_pool.tile([128, 2 * NQ], BF16)
                        nc.scalar.activation(
                            pe[:].rearrange("p (g q) -> p g q", g=2)[:, :, c0:NQ],
                            pss[:].rearrange("p (g q) -> p g q", g=2)[:, :, c0:NQ],
                            AF.Exp)
                        if r >= 0:
                            for hh in range(2):
                                nc.gpsimd.tensor_mul(
                                    pe[:, hh * NQ + c0:hh * NQ + c0 + 128],
                                    pe[:, hh * NQ + c0:hh * NQ + c0 + 128],
                                    tri[:])
                        pes.append((j, pe, c0))
                    for j, pe, c0 in pes:
                        for hh in range(2):
                            nc.tensor.matmul(
                                psa[hh][:, c0:NQ],
                                lhsT=v_sb[:, j, 2 * p + hh, :],
                                rhs=pe[:, hh * NQ + c0:hh * NQ + NQ],
                                start=(j == 0), stop=(j == nblk - 1))
                    # paced filler so the PE never drains at seams
                    if m <= 1:
                        pop_fill(2)
                    elif m == 2:
                        pop_fill(1)
                    else:
                        if gidx < 4 or (gidx % 2 == 1):
                            pop_fill(1)
                    gidx += 1
                # ---- per-pair finalize (all DVE; bcast+divide deferred) ----
                araw = [araw_pool.tile([65, NQ], FP32, name=f"araw{_hh}",
                                       tag=f"araw{_hh}") for _hh in range(2)]
                recb = rec_pool.tile([33, NQ], BF16, name="recb", tag="recb")
                nc.gpsimd.memset(recb[:], 0.0)
                for hh in range(2):
                    nc.vector.tensor_copy(araw[hh][:], psa[hh][0:65, :])
                with nc.allow_low_precision(reason="1/denom rounded to bf16"):
                    for hh in range(2):
                        nc.vector.reciprocal(recb[32 * hh:32 * hh + 1, :],
                                             araw[hh][64:65, :])
                last = (m == QM - 1 and p == NPAIR - 1)
                fb = make_fb(m, p, araw, recb, "ss" if last else "qkv")
                if last:
                    fb()
                else:
                    fill_q.appendleft(fb)
                if p == 0 and m + 1 < QM:
                    fill_q.append(qk_unit(m + 1, 0, 0))
                    fill_q.append(qk_unit(m + 1, 0, 1))
                    for st in range(4 * (m + 1), 4 * (m + 1) + 4):
                        fill_q.append(v_unit(st))
                    fill_q.append(qk_unit(m + 1, 1, 0))
                    fill_q.append(qk_unit(m + 1, 1, 1))
            fill_q.extend(proj_unit(m, t) for t in range(4))
        # drain any remaining filler (last macro's projection)
        while fill_q:
            fill_q.popleft()()

    nc.compile()
    return nc


_CACHED = {}


def _get_graph():
    if "nc" not in _CACHED:
        _CACHED["nc"] = build_graph()
    return _CACHED["nc"]


def make_in_maps(hidden_states, W_qkv, b_qkv, W_proj):
    bf16 = ml_dtypes.bfloat16
    in_maps = []
    xTb = [np.ascontiguousarray(hidden_states[b].T).astype(bf16)
           for b in range(B)]
    for c in range(NCORES):
        b = c // 4
        h0 = HPC * (c % 4)
        csl = slice(h0 * D, (h0 + HPC) * D)
        wq_s = np.ascontiguousarray(W_qkv[:, csl]).astype(bf16)
        wk_s = np.ascontiguousarray(W_qkv[:, E:][:, csl]).astype(bf16)
        wv_s = np.ascontiguousarray(W_qkv[:, 2 * E:][:, csl]).astype(bf16)
        bq = b_qkv[csl].reshape(HPC, D).T.astype(np.float32) / 8.0
        bk = b_qkv[E:][csl].reshape(HPC, D).T.astype(np.float32)
        qkbias = np.ascontiguousarray(
            np.concatenate([bq, bk], axis=0))          # (128, 4)
        wp_s = np.ascontiguousarray(W_proj[csl, :]).astype(bf16)
        in_maps.append({
            "xT": xTb[b], "wq": wq_s, "wk": wk_s, "wv": wv_s,
            "qkbias": qkbias, "wp": wp_s,
        })
    return in_maps


def kernel(hidden_states, W_qkv, b_qkv, W_proj, b_proj):
    from concourse.bass_utils import run_bass_kernel_spmd

    hidden_states = np.asarray(hidden_states, dtype=np.float32)
    W_qkv = np.asarray(W_qkv, dtype=np.float32)
    b_qkv = np.asarray(b_qkv, dtype=np.float32)
    W_proj = np.asarray(W_proj, dtype=np.float32)
    b_proj = np.asarray(b_proj, dtype=np.float32)

    nc = _get_graph()
    in_maps = make_in_maps(hidden_states, W_qkv, b_qkv, W_proj)
    res = None
    for attempt in range(3):
        try:
            res = run_bass_kernel_spmd(nc, in_maps, list(range(NCORES)))
            break
        except Exception:
            if attempt == 2:
                raise
            import time
            time.sleep(30 * (attempt + 1))
    partials = [res.results[c]["out"] for c in range(NCORES)]
    # V-bias folded out of the device kernel: softmax rows sum to one, so
    # the bias contributes bv @ W_proj to every output row exactly once.
    bias_eff = b_proj.astype(np.float64) + (
        b_qkv[2 * E:].astype(np.float64) @ W_proj.astype(np.float64))
    outp = np.empty((B, S, E), dtype=np.float32)
    for b in range(B):
        acc = np.zeros((S, E), dtype=np.float64)
        for c in range(4 * b, 4 * b + 4):
            acc += np.asarray(partials[c]).astype(np.float64)
        outp[b] = (acc + bias_eff).astype(np.float32)
    return outp


# revision 9
# speedup vs baseline: 1.0597x; 1.0597x over previous
"""Causal multi-head attention on 8 Trainium2 NeuronCores.

Problem: B=2, S=2048, E=1024, H=16 heads, D=64.
Sharding: core c handles batch b = c // 4 and heads [4*(c%4) .. 4*(c%4)+3]
(data parallel on B x tensor parallel on heads). Each core computes its
partial output projection in bf16; the host sums the 4 partials per batch
and adds b_proj (standard row-parallel TP reduction, done on host in f64).

Per-core kernel (all matmuls bf16 inputs, fp32 PSUM accumulation), computed
in "transposed" space to avoid transposing softmax probabilities:
  xT = X[b]^T in SBUF.
  Q^T/K^T [D, S] per head via col-packed matmuls (Q in psum rows 0:64, K in
  rows 64:128), evicted with bias (+1/8 scaling folded into Q) on DVE.
  V [S, D] per head with a ones column appended -> AV matmul also
  accumulates softmax denominators in psum row 64.
  Scores S^T [k, q] = K^T.T @ Q^T, row-packed across head pairs; emitted in
  two-block groups so consecutive score pairs share the K=64 PE config.
  P^T = exp(S^T) on ACT; diagonal blocks masked with a precomputed causal
  strip mask on gpsimd; strictly-upper blocks skipped.
  A^T [65, q] = V_ext.T @ P^T accumulated over k tiles (row 64 = denom).
  Per-pair finalize: evict raw A^T+denom (ACT for head 0, DVE for head 1),
  reciprocal into bf16 rows 0/32 of a [33,q] tile, one K=33 matmul
  broadcasts both heads' 1/denom across partitions, DVE tensor_mul divides
  into pair-stacked A_scaled^T.  Pair 0's chain overlaps pair 1's j-loop.
  partial[q, e] = A_scaled^T.T @ W_proj_rows accumulated over both pairs,
  with the stationary operand reused across both 512-wide e halves;
  evicted bf16 and DMA'd out.
Filler (next macro's QKV units, previous macro's projection) is emitted in
dumps at pair seams; the Tile scheduler interleaves it into the
exp-paced j-loops.
"""

import os
import sys
from contextlib import ExitStack

for _p in ("/opt/trn_rl_repo", "/root/.axon_site/_ro/trn_rl_repo"):
    if os.path.isdir(_p) and _p not in sys.path:
        sys.path.append(_p)

import numpy as np
import ml_dtypes

import concourse.bass as bass
import concourse.tile as tile
from concourse import bacc
from concourse import mybir
from concourse.masks import make_identity  # noqa: F401  (import check)

FP32 = mybir.dt.float32
BF16 = mybir.dt.bfloat16
AF = mybir.ActivationFunctionType

B, S, E, H = 2, 2048, 1024, 16
D = E // H          # 64
NCORES = 8
HPC = 4             # heads per core
NPAIR = 2           # head pairs per core
KT = E // 128       # 8 contraction tiles over E
ST = S // 128       # 16 tiles over S (k dimension)
QM = S // 512       # 4 q-macro tiles of 512
NQ = 512


def build_graph():
    nc = bacc.Bacc()

    xT = nc.declare_dram_parameter("xT", [E, S], BF16, isOutput=False)
    wq = nc.declare_dram_parameter("wq", [E, HPC * D], BF16, isOutput=False)
    wk = nc.declare_dram_parameter("wk", [E, HPC * D], BF16, isOutput=False)
    wv = nc.declare_dram_parameter("wv", [E, HPC * D], BF16, isOutput=False)
    qkbias = nc.declare_dram_parameter("qkbias", [128, HPC], FP32, isOutput=False)
    wp = nc.declare_dram_parameter("wp", [HPC * D, E], BF16, isOutput=False)
    out = nc.declare_dram_parameter("out", [S, E], BF16, isOutput=True)

    with tile.TileContext(nc) as tc, ExitStack() as ctx:
        const = ctx.enter_context(tc.tile_pool(name="const", bufs=1))
        sb = ctx.enter_context(tc.tile_pool(name="sb", bufs=1))
        pexp_pool = ctx.enter_context(tc.tile_pool(name="pexp", bufs=10))
        stage = ctx.enter_context(tc.tile_pool(name="stage", bufs=4))
        rec_pool = ctx.enter_context(tc.tile_pool(name="rec", bufs=2))
        araw_pool = ctx.enter_context(tc.tile_pool(name="araw", bufs=2))

        # PSUM budget is 8 banks: scores 2x2-bank + qkv/proj/bcast 2 + psa 2
        ps_s = ctx.enter_context(tc.tile_pool(name="ps_s", bufs=2, space="PSUM"))
        ps_qkv = ctx.enter_context(tc.tile_pool(name="ps_qkv", bufs=2, space="PSUM"))
        ps_a = ctx.enter_context(tc.tile_pool(name="ps_a", bufs=1, space="PSUM"))

        # ---- persistent SBUF tensors ----
        xt_sb = sb.tile([128, KT, S], BF16)          # X^T tiles, kt-major
        qt_sb = sb.tile([128, NPAIR, S], BF16)       # Q^T, pair-stacked
        kt_sb = sb.tile([128, NPAIR, S], BF16)       # K^T, pair-stacked
        v_sb = sb.tile([128, ST, HPC, D + 1], BF16)  # [V | ones] per ktile/head
        as_sb = sb.tile([128, NPAIR, S], BF16)       # A_scaled^T, pair-stacked
        wq_sb = sb.tile([128, KT, HPC * D], BF16)
        wk_sb = sb.tile([128, KT, HPC * D], BF16)
        wv_sb = sb.tile([128, KT, HPC * D], BF16)
        wp_sb = sb.tile([128, NPAIR, E], BF16)
        qkb_sb = const.tile([128, HPC], FP32)
        ones33 = const.tile([33, 128], BF16)         # K=33 lhsT: bcast pair
        tri = const.tile([128, 128], BF16)           # upper-tri (incl diag) strip mask

        # ---- constants ----
        # ones33: row 0 -> head 0's output partitions 0:64, row 32 -> head
        # 1's partitions 64:128 (32-aligned partition bases only).
        nc.any.memset(ones33[:], 0.0)
        nc.any.memset(ones33[0:1, 0:64], 1.0)
        nc.any.memset(ones33[32:33, 64:128], 1.0)
        nc.any.memset(v_sb[:, :, :, D:D + 1], 1.0)
        # PE warm-up: dummy matmuls on a zero tile while input DMAs land.
        warm = const.tile([128, NQ], BF16)
        nc.vector.memset(warm[:], 0.0)
        psw = ps_s.tile([128, 2 * NQ], FP32, name="psw", tag="ss")
        for _w in range(12):
            nc.tensor.matmul(psw[:, 0:NQ], lhsT=warm[:, 0:128], rhs=warm[:],
                             start=(_w == 0), stop=(_w == 11))
        # tri[kk, qq] = 1 where kk <= qq else 0
        nc.any.memset(tri[:], 1.0)
        nc.gpsimd.affine_select(
            out=tri[:], in_=tri[:],
            compare_op=mybir.AluOpType.is_ge, fill=0.0,
            base=0, pattern=[[1, 128]], channel_multiplier=-1)

        # ---- input DMAs: critical-first, alternating the two HWDGE rings ----
        _dq = [nc.sync, nc.scalar]
        _di = [0]

        def dma_in(dst, src):
            _dq[_di[0] % 2].dma_start(dst, src)
            _di[0] += 1

        for kt in range(KT):
            dma_in(wq_sb[:, kt, :], wq[kt * 128:(kt + 1) * 128, :])
            dma_in(xt_sb[:, kt, 0:NQ], xT[kt * 128:(kt + 1) * 128, 0:NQ])
        dma_in(qkb_sb[:], qkbias[:])
        for kt in range(KT):
            dma_in(wk_sb[:, kt, :], wk[kt * 128:(kt + 1) * 128, :])
        for kt in range(KT):
            dma_in(wv_sb[:, kt, :], wv[kt * 128:(kt + 1) * 128, :])
        for kt in range(KT):
            dma_in(xt_sb[:, kt, NQ:2 * NQ], xT[kt * 128:(kt + 1) * 128, NQ:2 * NQ])
        dma_in(wp_sb[:, 0, :], wp[0:128, :])
        dma_in(wp_sb[:, 1, :], wp[128:256, :])
        for _c in range(2, QM):
            for kt in range(KT):
                dma_in(xt_sb[:, kt, _c * NQ:(_c + 1) * NQ],
                       xT[kt * 128:(kt + 1) * 128, _c * NQ:(_c + 1) * NQ])

        # ---- filler unit builders ----
        def v_unit(st):
            def emit():
                psv = ps_qkv.tile([128, NQ], FP32, name="psv", tag="qkv")
                ssl = slice(st * 128, (st + 1) * 128)
                for kt in range(KT):
                    nc.tensor.matmul(
                        psv[:, 0:HPC * D], lhsT=xt_sb[:, kt, ssl],
                        rhs=wv_sb[:, kt, :], start=(kt == 0),
                        stop=(kt == KT - 1))
                nc.vector.tensor_copy(
                    v_sb[:, st, :, 0:D],
                    psv[:, 0:HPC * D].rearrange("p (h d) -> p h d", h=HPC))
            return emit

        def qk_unit(mm, p, hh):
            # Q/K alternate col groups per k-tile: adjacent MMs overlap
            def emit():
                msl = slice(mm * NQ, (mm + 1) * NQ)
                h = 2 * p + hh
                lo, hi = hh * 64, hh * 64 + 64
                psqk = ps_qkv.tile([128, NQ], FP32, name="psqk", tag="qkv")
                for kt in range(KT):
                    nc.tensor.matmul(
                        psqk[0:64, :],
                        lhsT=wq_sb[:, kt, h * D:(h + 1) * D],
                        rhs=xt_sb[:, kt, msl],
                        start=(kt == 0), stop=(kt == KT - 1),
                        tile_position=(0, 0), skip_group_check=True)
                    nc.tensor.matmul(
                        psqk[64:128, :],
                        lhsT=wk_sb[:, kt, h * D:(h + 1) * D],
                        rhs=xt_sb[:, kt, msl],
                        start=(kt == 0), stop=(kt == KT - 1),
                        tile_position=(0, 64), skip_group_check=True)
                nc.vector.tensor_scalar(
                    qt_sb[lo:hi, p, msl], psqk[0:64, :],
                    0.125, qkb_sb[0:64, h:h + 1],
                    op0=mybir.AluOpType.mult, op1=mybir.AluOpType.add)
                nc.vector.tensor_scalar_add(
                    kt_sb[lo:hi, p, msl], psqk[64:128, :],
                    qkb_sb[64:128, h:h + 1])
            return emit

        def proj_unit(mm, t):
            # stationary operand (as_sb pair block) reused for both e halves
            def emit():
                tsl = slice(mm * NQ + t * 128, mm * NQ + (t + 1) * 128)
                pso = [ps_qkv.tile([128, 512], FP32, name=f"pso{e}", tag="qkv")
                       for e in range(2)]
                for pp in range(NPAIR):
                    for e in range(2):
                        nc.tensor.matmul(
                            pso[e][:], lhsT=as_sb[:, pp, tsl],
                            rhs=wp_sb[:, pp, e * 512:(e + 1) * 512],
                            start=(pp == 0), stop=(pp == NPAIR - 1))
                for e in range(2):
                    osb = stage.tile([128, 512], BF16)
                    nc.vector.tensor_copy(osb[:], pso[e][:])
                    nc.sync.dma_start(out[tsl, e * 512:(e + 1) * 512], osb[:])
            return emit

        def make_fb(m, p, araw, recb, psb_tag):
            # broadcast 1/denom for both heads with one K=33 matmul, divide
            def emit():
                msl = slice(m * NQ, (m + 1) * NQ)
                psb = ps_qkv.tile([128, NQ], FP32, name="psb", tag=psb_tag) \
                    if psb_tag == "qkv" else \
                    ps_s.tile([128, NQ], FP32, name="psbs", tag="ss")
                nc.tensor.matmul(psb[:, :], lhsT=ones33[:], rhs=recb[0:33, :],
                                 start=True, stop=True)
                for hh in range(2):
                    lo, hi = hh * 64, hh * 64 + 64
                    nc.vector.tensor_mul(
                        as_sb[lo:hi, p, msl], psb[lo:hi, :], araw[hh][0:64, :])
            return emit

        def finalize_a(m, p, psa):
            # evict raw A^T + denom; head 0 via ACT, head 1 via DVE (parallel)
            araw = [araw_pool.tile([65, NQ], FP32, name=f"araw{_hh}",
                                   tag=f"araw{_hh}") for _hh in range(2)]
            recb = rec_pool.tile([33, NQ], BF16, name="recb", tag="recb")
            nc.gpsimd.memset(recb[:], 0.0)
            nc.scalar.copy(araw[0][:], psa[0][0:65, :])
            nc.vector.tensor_copy(araw[1][:], psa[1][0:65, :])
            with nc.allow_low_precision(reason="1/denom rounded to bf16"):
                for hh in range(2):
                    nc.vector.reciprocal(recb[32 * hh:32 * hh + 1, :],
                                         araw[hh][64:65, :])
            return araw, recb

        # ---- macro 0 prologue: pair-0 QK while DMAs land, then V 0..3 ----
        qk_unit(0, 0, 0)()
        qk_unit(0, 0, 1)()
        for st in range(4):
            v_unit(st)()
        qk_unit(0, 1, 0)()
        qk_unit(0, 1, 1)()

        proj_stash = []
        fb_stash = []
        for m in range(QM):
            nblk = 4 * m + 4
            for p in range(NPAIR):
                if p == 1:
                    # previous macro's projection: PE filler at the seam
                    while proj_stash:
                        proj_stash.pop(0)()
                psa = [ps_a.tile([65, NQ], FP32, name=f"psa{_hh}",
                                 tag=f"psa{_hh}") for _hh in range(2)]
                for jj in range(0, nblk, 2):
                    infos = []
                    for j in (jj, jj + 1):
                        jsl = slice(j * 128, (j + 1) * 128)
                        r = j - 4 * m
                        c0 = 128 * r if r > 0 else 0
                        pss = ps_s.tile([128, 2 * NQ], FP32, name="pss",
                                        tag="ss")
                        for hh in range(2):
                            lo, hi = hh * 64, hh * 64 + 64
                            nc.tensor.matmul(
                                pss[:, hh * NQ + c0:hh * NQ + NQ],
                                lhsT=kt_sb[lo:hi, p, jsl],
                                rhs=qt_sb[lo:hi, p,
                                          m * NQ + c0:m * NQ + NQ],
                                start=True, stop=True,
                                tile_position=(hh * 64, 0))
                        infos.append((j, pss, c0, r))
                    pes = []
                    for j, pss, c0, r in infos:
                        pe = pexp_pool.tile([128, 2 * NQ], BF16)
                        nc.scalar.activation(
                            pe[:].rearrange("p (g q) -> p g q", g=2)[:, :, c0:NQ],
                            pss[:].rearrange("p (g q) -> p g q", g=2)[:, :, c0:NQ],
                            AF.Exp)
                        if r >= 0:
                            for hh in range(2):
                                nc.gpsimd.tensor_mul(
                                    pe[:, hh * NQ + c0:hh * NQ + c0 + 128],
                                    pe[:, hh * NQ + c0:hh * NQ + c0 + 128],
                                    tri[:])
                        pes.append((j, pe, c0))
                    for j, pe, c0 in pes:
                        for hh in range(2):
                            nc.tensor.matmul(
                                psa[hh][:, c0:NQ],
                                lhsT=v_sb[:, j, 2 * p + hh, :],
                                rhs=pe[:, hh * NQ + c0:hh * NQ + NQ],
                                start=(j == 0), stop=(j == nblk - 1))
                # per-pair finalize chain; bcast+divide deferred for overlap
                araw, recb = finalize_a(m, p, psa)
                last = (m == QM - 1 and p == NPAIR - 1)
                if p == 0:
                    # pair 0's chain completes under pair 1's j-loop
                    fb_stash.append(make_fb(m, p, araw, recb, "qkv"))
                else:
                    fb1 = make_fb(m, p, araw, recb, "ss" if last else "qkv")
            # pair0's bcast+divide: chain long done, runs immediately
            while fb_stash:
                fb_stash.pop(0)()
            if m + 1 < QM:
                # filler dump: Tile interleaves into the exp-paced loops
                for st in range(4 * (m + 1), 4 * (m + 1) + 4):
                    v_unit(st)()
                for pp in range(NPAIR):
                    for hh in range(2):
                        qk_unit(m + 1, pp, hh)()
                fb1()
                proj_stash = [proj_unit(m, t) for t in range(4)]
            else:
                # tail: cover pair 1's chain with the last stashed proj units
                while proj_stash:
                    proj_stash.pop(0)()
                fb1()
                for t in range(4):
                    proj_unit(m, t)()

    nc.compile()
    return nc


_CACHED = {}


def _get_graph():
    if "nc" not in _CACHED:
        _CACHED["nc"] = build_graph()
    return _CACHED["nc"]


def make_in_maps(hidden_states, W_qkv, b_qkv, W_proj):
    bf16 = ml_dtypes.bfloat16
    in_maps = []
    xTb = [np.ascontiguousarray(hidden_states[b].T).astype(bf16)
           for b in range(B)]
    for c in range(NCORES):
        b = c // 4
        h0 = HPC * (c % 4)
        csl = slice(h0 * D, (h0 + HPC) * D)
        wq_s = np.ascontiguousarray(W_qkv[:, csl]).astype(bf16)
        wk_s = np.ascontiguousarray(W_qkv[:, E:][:, csl]).astype(bf16)
        wv_s = np.ascontiguousarray(W_qkv[:, 2 * E:][:, csl]).astype(bf16)
        bq = b_qkv[csl].reshape(HPC, D).T.astype(np.float32) / 8.0
        bk = b_qkv[E:][csl].reshape(HPC, D).T.astype(np.float32)
        qkbias = np.ascontiguousarray(
            np.concatenate([bq, bk], axis=0))          # (128, 4)
        wp_s = np.ascontiguousarray(W_proj[csl, :]).astype(bf16)
        in_maps.append({
            "xT": xTb[b], "wq": wq_s, "wk": wk_s, "wv": wv_s,
            "qkbias": qkbias, "wp": wp_s,
        })
    return in_maps


def kernel(hidden_states, W_qkv, b_qkv, W_proj, b_proj):
    from concourse.bass_utils import run_bass_kernel_spmd

    hidden_states = np.asarray(hidden_states, dtype=np.float32)
    W_qkv = np.asarray(W_qkv, dtype=np.float32)
    b_qkv = np.asarray(b_qkv, dtype=np.float32)
    W_proj = np.asarray(W_proj, dtype=np.float32)
    b_proj = np.asarray(b_proj, dtype=np.float32)

    nc = _get_graph()
    in_maps = make_in_maps(hidden_states, W_qkv, b_qkv, W_proj)
    res = None
    for attempt in range(3):
        try:
            res = run_bass_kernel_spmd(nc, in_maps, list(range(NCORES)))
            break
        except Exception:
            if attempt == 2:
                raise
            import time
            time.sleep(30 * (attempt + 1))
    partials = [res.results[c]["out"] for c in range(NCORES)]
    # V-bias folded out of the device kernel: softmax rows sum to one, so
    # the bias contributes bv @ W_proj to every output row exactly once.
    bias_eff = b_proj.astype(np.float64) + (
        b_qkv[2 * E:].astype(np.float64) @ W_proj.astype(np.float64))
    outp = np.empty((B, S, E), dtype=np.float32)
    for b in range(B):
        acc = np.zeros((S, E), dtype=np.float64)
        for c in range(4 * b, 4 * b + 4):
            acc += np.asarray(partials[c]).astype(np.float64)
        outp[b] = (acc + bias_eff).astype(np.float32)
    return outp


# revision 16
# speedup vs baseline: 1.3195x; 1.2452x over previous
"""Causal multi-head attention on 8 Trainium2 NeuronCores.

Problem: B=2, S=2048, E=1024, H=16 heads, D=64.
Sharding: core c handles batch b = c // 4 and heads [4*(c%4) .. 4*(c%4)+3]
(data parallel on B x tensor parallel on heads). Each core computes its
partial output projection in bf16; the host sums the 4 partials per batch
and adds b_proj (standard row-parallel TP reduction, done on host in f64).

Per-core kernel (all matmuls bf16 inputs, fp32 PSUM accumulation), computed
in "transposed" space to avoid transposing softmax probabilities:
  xT = X[b]^T in SBUF.
  Q^T/K^T [D, S] per head via col-packed matmuls (Q in psum rows 0:64, K in
  rows 64:128), evicted with bias (+1/8 scaling folded into Q) on DVE.
  V [S, D] per head with a ones column appended -> AV matmul also
  accumulates softmax denominators in psum row 64.
  Scores S^T [k, q] = K^T.T @ Q^T, row-packed across head pairs; emitted in
  two-block groups so consecutive score pairs share the K=64 PE config.
  P^T = exp(S^T) on ACT; diagonal blocks masked with a precomputed causal
  strip mask on gpsimd; strictly-upper blocks skipped.
  A^T [65, q] = V_ext.T @ P^T accumulated over k tiles (row 64 = denom).
  Per-pair finalize: evict raw A^T+denom (ACT for head 0, DVE for head 1),
  reciprocal into bf16 rows 0/32 of a [33,q] tile, one K=33 matmul
  broadcasts both heads' 1/denom across partitions, DVE tensor_mul divides
  into pair-stacked A_scaled^T.  Pair 0's chain overlaps pair 1's j-loop.
  partial[q, e] = A_scaled^T.T @ W_proj_rows accumulated over both pairs,
  with the stationary operand reused across both 512-wide e halves;
  evicted bf16 and DMA'd out.
Filler (next macro's QKV units, previous macro's projection) is emitted in
dumps at pair seams; the Tile scheduler interleaves it into the
exp-paced j-loops.
"""

import os
import sys
from contextlib import ExitStack

for _p in ("/opt/trn_rl_repo", "/root/.axon_site/_ro/trn_rl_repo"):
    if os.path.isdir(_p) and _p not in sys.path:
        sys.path.append(_p)

import numpy as np
import ml_dtypes

import concourse.bass as bass
import concourse.tile as tile
from concourse import bacc
from concourse import mybir
from concourse.masks import make_identity  # noqa: F401  (import check)

FP32 = mybir.dt.float32
BF16 = mybir.dt.bfloat16
AF = mybir.ActivationFunctionType

B, S, E, H = 2, 2048, 1024, 16
D = E // H          # 64
NCORES = 8
HPC = 4             # heads per core
NPAIR = 2           # head pairs per core
KT = E // 128       # 8 contraction tiles over E
ST = S // 128       # 16 tiles over S (k dimension)
QM = S // 512       # 4 q-macro tiles of 512
NQ = 512


def build_graph():
    nc = bacc.Bacc()

    xT = nc.declare_dram_parameter("xT", [E, S], BF16, isOutput=False)
    wq = nc.declare_dram_parameter("wq", [E, HPC * D], BF16, isOutput=False)
    wk = nc.declare_dram_parameter("wk", [E, HPC * D], BF16, isOutput=False)
    wv = nc.declare_dram_parameter("wv", [E, HPC * D], BF16, isOutput=False)
    qkbias = nc.declare_dram_parameter("qkbias", [128, HPC], FP32, isOutput=False)
    wp = nc.declare_dram_parameter("wp", [HPC * D, E], BF16, isOutput=False)
    out = nc.declare_dram_parameter("out", [S, E], BF16, isOutput=True)

    with tile.TileContext(nc) as tc, ExitStack() as ctx:
        const = ctx.enter_context(tc.tile_pool(name="const", bufs=1))
        sb = ctx.enter_context(tc.tile_pool(name="sb", bufs=1))
        pexp_pool = ctx.enter_context(tc.tile_pool(name="pexp", bufs=10))
        stage = ctx.enter_context(tc.tile_pool(name="stage", bufs=4))
        rec_pool = ctx.enter_context(tc.tile_pool(name="rec", bufs=2))
        araw_pool = ctx.enter_context(tc.tile_pool(name="araw", bufs=2))

        # PSUM budget is 8 banks: scores 2x2-bank + qkv/proj/bcast 2 + psa 2
        ps_s = ctx.enter_context(tc.tile_pool(name="ps_s", bufs=2, space="PSUM"))
        ps_qkv = ctx.enter_context(tc.tile_pool(name="ps_qkv", bufs=2, space="PSUM"))
        ps_a = ctx.enter_context(tc.tile_pool(name="ps_a", bufs=1, space="PSUM"))

        # ---- persistent SBUF tensors ----
        xt_sb = sb.tile([128, KT, S], BF16)          # X^T tiles, kt-major
        qt_sb = sb.tile([128, NPAIR, S], BF16)       # Q^T, pair-stacked
        kt_sb = sb.tile([128, NPAIR, S], BF16)       # K^T, pair-stacked
        # [ones | 63*0 | V] per ktile/head: AV psum row 0 = softmax denom
        # (partition base 0 for the approx-reciprocal custom op), rows 64:128
        # = A^T (engine partition accesses must not cross the 64 boundary).
        v_sb = sb.tile([128, ST, HPC, 128], BF16)
        as_sb = sb.tile([128, NPAIR, S], BF16)       # A_scaled^T, pair-stacked
        wq_sb = sb.tile([128, KT, HPC * D], BF16)
        wk_sb = sb.tile([128, KT, HPC * D], BF16)
        wv_sb = sb.tile([128, KT, HPC * D], BF16)
        wp_sb = sb.tile([128, NPAIR, E], BF16)
        qkb_sb = const.tile([128, HPC], FP32)
        ones1 = const.tile([1, 128], BF16)           # K=1 lhsT row for bcasts
        tri = const.tile([128, 128], BF16)           # upper-tri (incl diag) strip mask

        # ---- constants ----
        nc.any.memset(ones1[:], 1.0)
        nc.any.memset(v_sb[:, :, :, 0:1], 1.0)
        nc.any.memset(v_sb[:, :, :, 1:64], 0.0)
        # PE warm-up: dummy matmuls on a zero tile while input DMAs land.
        warm = const.tile([128, NQ], BF16)
        nc.vector.memset(warm[:], 0.0)
        psw = ps_s.tile([128, 2 * NQ], FP32, name="psw", tag="ss")
        for _w in range(12):
            nc.tensor.matmul(psw[:, 0:NQ], lhsT=warm[:, 0:128], rhs=warm[:],
                             start=(_w == 0), stop=(_w == 11))
        # tri[kk, qq] = 1 where kk <= qq else 0
        nc.any.memset(tri[:], 1.0)
        nc.gpsimd.affine_select(
            out=tri[:], in_=tri[:],
            compare_op=mybir.AluOpType.is_ge, fill=0.0,
            base=0, pattern=[[1, 128]], channel_multiplier=-1)

        # ---- input DMAs: critical-first, alternating the two HWDGE rings ----
        _dq = [nc.sync, nc.scalar]
        _di = [0]

        def dma_in(dst, src):
            _dq[_di[0] % 2].dma_start(dst, src)
            _di[0] += 1

        for kt in range(KT):
            dma_in(wq_sb[:, kt, :], wq[kt * 128:(kt + 1) * 128, :])
            dma_in(xt_sb[:, kt, 0:NQ], xT[kt * 128:(kt + 1) * 128, 0:NQ])
        dma_in(qkb_sb[:], qkbias[:])
        for kt in range(KT):
            dma_in(wk_sb[:, kt, :], wk[kt * 128:(kt + 1) * 128, :])
        for kt in range(KT):
            dma_in(wv_sb[:, kt, :], wv[kt * 128:(kt + 1) * 128, :])
        for kt in range(KT):
            dma_in(xt_sb[:, kt, NQ:2 * NQ], xT[kt * 128:(kt + 1) * 128, NQ:2 * NQ])
        dma_in(wp_sb[:, 0, :], wp[0:128, :])
        dma_in(wp_sb[:, 1, :], wp[128:256, :])
        for _c in range(2, QM):
            for kt in range(KT):
                dma_in(xt_sb[:, kt, _c * NQ:(_c + 1) * NQ],
                       xT[kt * 128:(kt + 1) * 128, _c * NQ:(_c + 1) * NQ])

        # ---- filler unit builders ----
        def v_unit(st):
            def emit():
                psv = ps_qkv.tile([128, NQ], FP32, name="psv", tag="qkv")
                ssl = slice(st * 128, (st + 1) * 128)
                for kt in range(KT):
                    nc.tensor.matmul(
                        psv[:, 0:HPC * D], lhsT=xt_sb[:, kt, ssl],
                        rhs=wv_sb[:, kt, :], start=(kt == 0),
                        stop=(kt == KT - 1))
                nc.vector.tensor_copy(
                    v_sb[:, st, :, 64:128],
                    psv[:, 0:HPC * D].rearrange("p (h d) -> p h d", h=HPC))
            return emit

        def qk_unit(mm, p, hh):
            # Q/K alternate col groups per k-tile: adjacent MMs overlap
            def emit():
                msl = slice(mm * NQ, (mm + 1) * NQ)
                h = 2 * p + hh
                lo, hi = hh * 64, hh * 64 + 64
                psqk = ps_qkv.tile([128, NQ], FP32, name="psqk", tag="qkv")
                for kt in range(KT):
                    nc.tensor.matmul(
                        psqk[0:64, :],
                        lhsT=wq_sb[:, kt, h * D:(h + 1) * D],
                        rhs=xt_sb[:, kt, msl],
                        start=(kt == 0), stop=(kt == KT - 1),
                        tile_position=(0, 0), skip_group_check=True)
                    nc.tensor.matmul(
                        psqk[64:128, :],
                        lhsT=wk_sb[:, kt, h * D:(h + 1) * D],
                        rhs=xt_sb[:, kt, msl],
                        start=(kt == 0), stop=(kt == KT - 1),
                        tile_position=(0, 64), skip_group_check=True)
                nc.vector.tensor_scalar(
                    qt_sb[lo:hi, p, msl], psqk[0:64, :],
                    0.125, qkb_sb[0:64, h:h + 1],
                    op0=mybir.AluOpType.mult, op1=mybir.AluOpType.add)
                nc.vector.tensor_scalar_add(
                    kt_sb[lo:hi, p, msl], psqk[64:128, :],
                    qkb_sb[64:128, h:h + 1])
            return emit

        def proj_unit(mm, t):
            # stationary operand (as_sb pair block) reused for both e halves
            def emit():
                tsl = slice(mm * NQ + t * 128, mm * NQ + (t + 1) * 128)
                pso = [ps_qkv.tile([128, 512], FP32, name=f"pso{e}", tag="qkv")
                       for e in range(2)]
                for pp in range(NPAIR):
                    for e in range(2):
                        nc.tensor.matmul(
                            pso[e][:], lhsT=as_sb[:, pp, tsl],
                            rhs=wp_sb[:, pp, e * 512:(e + 1) * 512],
                            start=(pp == 0), stop=(pp == NPAIR - 1))
                for e in range(2):
                    osb = stage.tile([128, 512], BF16)
                    nc.vector.tensor_copy(osb[:], pso[e][:])
                    nc.sync.dma_start(out[tsl, e * 512:(e + 1) * 512], osb[:])
            return emit

        def make_fb(m, p, araw, recb, psb_tag):
            # broadcast 1/denom across partitions: one K=1 matmul per head,
            # concurrent via distinct col-group array tiles; then divide
            def emit():
                msl = slice(m * NQ, (m + 1) * NQ)
                psb = ps_qkv.tile([128, NQ], FP32, name="psb", tag=psb_tag) \
                    if psb_tag == "qkv" else \
                    ps_s.tile([128, NQ], FP32, name="psbs", tag="ss")
                for hh in range(2):
                    lo, hi = hh * 64, hh * 64 + 64
                    nc.tensor.matmul(psb[lo:hi, :],
                                     lhsT=ones1[0:1, lo:hi],
                                     rhs=recb[hh][0:1, :],
                                     start=True, stop=True,
                                     tile_position=(0, lo),
                                     skip_group_check=True)
                for hh in range(2):
                    lo, hi = hh * 64, hh * 64 + 64
                    nc.vector.tensor_mul(
                        as_sb[lo:hi, p, msl], psb[lo:hi, :],
                        araw[hh][64:128, :])
            return emit

        def finalize_a(m, p, psa):
            # evict raw A^T + denom; head 0 via ACT, head 1 via DVE (parallel)
            araw = [araw_pool.tile([128, NQ], FP32, name=f"araw{_hh}",
                                   tag=f"araw{_hh}") for _hh in range(2)]
            rec32 = [rec_pool.tile([1, NQ], FP32, name=f"rec32_{_hh}",
                                   tag=f"rec32_{_hh}") for _hh in range(2)]
            recb = [rec_pool.tile([1, NQ], BF16, name=f"recb{_hh}",
                                  tag=f"recb{_hh}") for _hh in range(2)]
            nc.scalar.copy(araw[0][:], psa[0][0:128, :])
            nc.vector.tensor_copy(araw[1][:], psa[1][0:128, :])
            for hh in range(2):
                nc.vector.reciprocal_approx_fast(
                    out=rec32[hh][0:1, :], in_=araw[hh][0:1, :])
                nc.vector.tensor_copy(recb[hh][0:1, :], rec32[hh][0:1, :])
            return araw, recb

        # ---- macro 0 prologue: pair-0 QK while DMAs land, then V 0..3 ----
        qk_unit(0, 0, 0)()
        qk_unit(0, 0, 1)()
        for st in range(4):
            v_unit(st)()
        qk_unit(0, 1, 0)()
        qk_unit(0, 1, 1)()

        proj_stash = []
        fb_stash = []
        for m in range(QM):
            nblk = 4 * m + 4
            for p in range(NPAIR):
                if p == 1:
                    # previous macro's projection: PE filler at the seam
                    while proj_stash:
                        proj_stash.pop(0)()
                psa = [ps_a.tile([128, NQ], FP32, name=f"psa{_hh}",
                                 tag=f"psa{_hh}") for _hh in range(2)]
                for jj in range(0, nblk, 2):
                    infos = []
                    for j in (jj, jj + 1):
                        jsl = slice(j * 128, (j + 1) * 128)
                        r = j - 4 * m
                        c0 = 128 * r if r > 0 else 0
                        pss = ps_s.tile([128, 2 * NQ], FP32, name="pss",
                                        tag="ss")
                        for hh in range(2):
                            lo, hi = hh * 64, hh * 64 + 64
                            nc.tensor.matmul(
                                pss[:, hh * NQ + c0:hh * NQ + NQ],
                                lhsT=kt_sb[lo:hi, p, jsl],
                                rhs=qt_sb[lo:hi, p,
                                          m * NQ + c0:m * NQ + NQ],
                                start=True, stop=True,
                                tile_position=(hh * 64, 0))
                        infos.append((j, pss, c0, r))
                    pes = []
                    for j, pss, c0, r in infos:
                        pe = pexp_pool.tile([128, 2 * NQ], BF16)
                        nc.scalar.activation(
                            pe[:].rearrange("p (g q) -> p g q", g=2)[:, :, c0:NQ],
                            pss[:].rearrange("p (g q) -> p g q", g=2)[:, :, c0:NQ],
                            AF.Exp)
                        if r >= 0:
                            for hh in range(2):
                                nc.gpsimd.tensor_mul(
                                    pe[:, hh * NQ + c0:hh * NQ + c0 + 128],
                                    pe[:, hh * NQ + c0:hh * NQ + c0 + 128],
                                    tri[:])
                        pes.append((j, pe, c0))
                    for j, pe, c0 in pes:
                        for hh in range(2):
                            nc.tensor.matmul(
                                psa[hh][:, c0:NQ],
                                lhsT=v_sb[:, j, 2 * p + hh, :],
                                rhs=pe[:, hh * NQ + c0:hh * NQ + NQ],
                                start=(j == 0), stop=(j == nblk - 1))
                # per-pair finalize chain; bcast+divide deferred for overlap
                araw, recb = finalize_a(m, p, psa)
                last = (m == QM - 1 and p == NPAIR - 1)
                if p == 0:
                    # pair 0's chain completes under pair 1's j-loop
                    fb_stash.append(make_fb(m, p, araw, recb, "qkv"))
                else:
                    fb1 = make_fb(m, p, araw, recb, "ss" if last else "qkv")
            # pair0's bcast+divide: chain long done, runs immediately
            while fb_stash:
                fb_stash.pop(0)()
            if m + 1 < QM:
                # filler dump: Tile interleaves into the exp-paced loops
                for st in range(4 * (m + 1), 4 * (m + 1) + 4):
                    v_unit(st)()
                for pp in range(NPAIR):
                    for hh in range(2):
                        qk_unit(m + 1, pp, hh)()
                fb1()
                proj_stash = [proj_unit(m, t) for t in range(4)]
            else:
                # tail: cover pair 1's chain with the last stashed proj units
                while proj_stash:
                    proj_stash.pop(0)()
                fb1()
                for t in range(4):
                    proj_unit(m, t)()

    nc.compile()
    return nc


_CACHED = {}


def _get_graph():
    if "nc" not in _CACHED:
        _CACHED["nc"] = build_graph()
    return _CACHED["nc"]


def make_in_maps(hidden_states, W_qkv, b_qkv, W_proj):
    bf16 = ml_dtypes.bfloat16
    in_maps = []
    xTb = [np.ascontiguousarray(hidden_states[b].T).astype(bf16)
           for b in range(B)]
    for c in range(NCORES):
        b = c // 4
        h0 = HPC * (c % 4)
        csl = slice(h0 * D, (h0 + HPC) * D)
        wq_s = np.ascontiguousarray(W_qkv[:, csl]).astype(bf16)
        wk_s = np.ascontiguousarray(W_qkv[:, E:][:, csl]).astype(bf16)
        wv_s = np.ascontiguousarray(W_qkv[:, 2 * E:][:, csl]).astype(bf16)
        bq = b_qkv[csl].reshape(HPC, D).T.astype(np.float32) / 8.0
        bk = b_qkv[E:][csl].reshape(HPC, D).T.astype(np.float32)
        qkbias = np.ascontiguousarray(
            np.concatenate([bq, bk], axis=0))          # (128, 4)
        wp_s = np.ascontiguousarray(W_proj[csl, :]).astype(bf16)
        in_maps.append({
            "xT": xTb[b], "wq": wq_s, "wk": wk_s, "wv": wv_s,
            "qkbias": qkbias, "wp": wp_s,
        })
    return in_maps


def kernel(hidden_states, W_qkv, b_qkv, W_proj, b_proj):
    from concourse.bass_utils import run_bass_kernel_spmd

    hidden_states = np.asarray(hidden_states, dtype=np.float32)
    W_qkv = np.asarray(W_qkv, dtype=np.float32)
    b_qkv = np.asarray(b_qkv, dtype=np.float32)
    W_proj = np.asarray(W_proj, dtype=np.float32)
    b_proj = np.asarray(b_proj, dtype=np.float32)

    nc = _get_graph()
    in_maps = make_in_maps(hidden_states, W_qkv, b_qkv, W_proj)
    res = None
    for attempt in range(3):
        try:
            res = run_bass_kernel_spmd(nc, in_maps, list(range(NCORES)))
            break
        except Exception:
            if attempt == 2:
                raise
            import time
            time.sleep(30 * (attempt + 1))
    partials = [res.results[c]["out"] for c in range(NCORES)]
    # V-bias folded out of the device kernel: softmax rows sum to one, so
    # the bias contributes bv @ W_proj to every output row exactly once.
    bias_eff = b_proj.astype(np.float64) + (
        b_qkv[2 * E:].astype(np.float64) @ W_proj.astype(np.float64))
    outp = np.empty((B, S, E), dtype=np.float32)
    for b in range(B):
        acc = np.zeros((S, E), dtype=np.float64)
        for c in range(4 * b, 4 * b + 4):
            acc += np.asarray(partials[c]).astype(np.float64)
        outp[b] = (acc + bias_eff).astype(np.float32)
    return outp
